# revision 7
# baseline (speedup 1.0000x reference)
# DETR multi-head dot-product attention for Trainium2 (Bass/Tile), 8 NeuronCores.
#
# Problem (hardcoded): B=4, S=1024, D=1024, H=16, HD=64, f32.
#   q = (inputs_q + pos_emb_q) @ wq + bq;  q /= sqrt(HD)
#   k = (inputs_kv + pos_emb_k) @ wk + bk
#   v = (inputs_kv + pos_emb_v) @ wv + bv          (bv == 0 by problem spec)
#   attn = softmax(q k^T + key_padding_bias); out = (attn v) @ wo + bo
#
# Sharding: 8 cores = 4 batches x 2 head-groups of 8 heads. Each core computes
# its batch's projections restricted to its head-group's features (512 of 1024),
# full attention for its 8 heads, and a partial output projection. The host
# sums the two head-group partials per batch.
#
# Layout: activations are kept feature-major ("transposed", [D, S]) on device;
# the host ships inputs pre-transposed so no on-device transposes are needed.
# Matmul convention: out[M,N] = lhsT[K,M].T @ rhs[K,N], contraction over the
# partition dim K. Softmax runs over the partition axis of transposed logits
# L^T[S_k, S_q]; the row-sums come for free from a ones-column appended to V.
# All matmuls run in float32r (TF32-like, 4x faster than fp32 on the PE).

import sys

for _p in ("/opt/trn_rl_repo", "/root/.axon_site/_ro/trn_rl_repo"):
    if _p not in sys.path:
        sys.path.append(_p)

import numpy as np

import concourse.bass as bass
import concourse.mybir as mybir
import concourse.tile as tile
from concourse import bacc
from concourse.bass_utils import run_bass_kernel_spmd

B, S, D = 4, 1024, 1024
H, HD = 16, 64
F = 512          # features per head-group core (8 heads * 64)
NH = 8           # heads per core
NEG_BIG = -1e10
P = 128          # partitions
KC = D // P      # contraction chunks for the input projections (8)
SC = S // P      # sequence chunks (8)
SH = 512         # S-half (moving-operand free dim for f32r matmuls)

f32 = mybir.dt.float32
f32r = mybir.dt.float32r


def build_program():
    nc = bacc.Bacc("TRN2", target_bir_lowering=False, debug=False)

    xq_d = nc.dram_tensor("xq", [D, S], f32, kind="ExternalInput")
    xkv_d = nc.dram_tensor("xkv", [D, S], f32, kind="ExternalInput")
    pq_d = nc.dram_tensor("pq", [D, S], f32, kind="ExternalInput")
    pk_d = nc.dram_tensor("pk", [D, S], f32, kind="ExternalInput")
    pv_d = nc.dram_tensor("pv", [D, S], f32, kind="ExternalInput")
    wq_d = nc.dram_tensor("wq", [D, F], f32r, kind="ExternalInput")
    wk_d = nc.dram_tensor("wk", [D, F], f32r, kind="ExternalInput")
    wv_d = nc.dram_tensor("wv", [D, F], f32r, kind="ExternalInput")
    wo_d = nc.dram_tensor("wo", [F, D], f32r, kind="ExternalInput")
    bq_d = nc.dram_tensor("bq", [F], f32, kind="ExternalInput")
    bk_d = nc.dram_tensor("bk", [F], f32, kind="ExternalInput")
    bo_d = nc.dram_tensor("bo", [D], f32, kind="ExternalInput")
    mb_d = nc.dram_tensor("mb", [S], f32, kind="ExternalInput")
    # memset into float32r tiles fails the walrus ISA check; ship ones instead
    vones_d = nc.dram_tensor("vones", [P, SC, NH], f32r, kind="ExternalInput")
    ones_d = nc.dram_tensor("ones", [1, HD], f32r, kind="ExternalInput")
    out_d = nc.dram_tensor("out_t", [D, S], f32, kind="ExternalOutput")

    with tile.TileContext(nc) as tc:
        with (
            tc.tile_pool(name="raw", bufs=6) as raw_pool,
            tc.tile_pool(name="acts", bufs=3) as acts_pool,
            tc.tile_pool(name="wmat", bufs=2) as w_pool,
            tc.tile_pool(name="persist", bufs=1) as persist,
            tc.tile_pool(name="pbuf", bufs=2) as p_pool,
            tc.tile_pool(name="small", bufs=2) as small,
            tc.tile_pool(name="outb", bufs=2) as out_pool,
            tc.tile_pool(name="ps", bufs=8, space=bass.MemorySpace.PSUM) as ps,
        ):
            # ---- persistent tiles ----
            qt = persist.tile([P, 4, S], f32r, tag="qt")     # Q^T  [feature, s]
            kt = persist.tile([P, 4, S], f32r, tag="kt")     # K^T  [feature, s]
            xt = persist.tile([P, 4, S], f32r, tag="xt")     # attn-out^T, normalized
            # V in natural layout [s, head, hd] with a ones column per head.
            vsb = persist.tile([P, SC, NH, HD + 1], f32r, tag="vsb")
            bq_sb = persist.tile([P, 4], f32, tag="bq")
            bk_sb = persist.tile([P, 4], f32, tag="bk")
            bo_sb = persist.tile([P, KC], f32, tag="bo")
            mb_sb = persist.tile([P, SC], f32, tag="mb")
            ones_sb = persist.tile([1, HD], f32r, tag="ones")

            nc.sync.dma_start(vsb[:, :, :, HD], vones_d[:])
            nc.sync.dma_start(ones_sb[:], ones_d[:])
            nc.sync.dma_start(bq_sb[:], bq_d[:].rearrange("(m p) -> p m", p=P))
            nc.sync.dma_start(bk_sb[:], bk_d[:].rearrange("(m p) -> p m", p=P))
            nc.sync.dma_start(bo_sb[:], bo_d[:].rearrange("(m p) -> p m", p=P))
            nc.sync.dma_start(mb_sb[:], mb_d[:].rearrange("(c p) -> p c", p=P))

            wq_sb = w_pool.tile([P, KC, F], f32r, tag="w")
            nc.sync.dma_start(wq_sb[:], wq_d[:].rearrange("(k p) f -> p k f", p=P))

            # ---- Q path: qin^T = xq^T + pq^T, then Q^T = (wq^T qin^T) + bq ----
            for sh in range(2):
                qin = acts_pool.tile([P, KC, SH], f32r, tag="acts")
                for c in range(KC):
                    xr = raw_pool.tile([P, SH], f32, tag="raw")
                    pr = raw_pool.tile([P, SH], f32, tag="raw")
                    nc.sync.dma_start(
                        xr[:], xq_d[c * P:(c + 1) * P, sh * SH:(sh + 1) * SH])
                    nc.sync.dma_start(
                        pr[:], pq_d[c * P:(c + 1) * P, sh * SH:(sh + 1) * SH])
                    nc.vector.tensor_add(qin[:, c, :], xr[:], pr[:])
                for m in range(4):
                    acc = ps.tile([P, SH], f32, tag="ps")
                    for k in range(KC):
                        nc.tensor.matmul(
                            acc[:],
                            wq_sb[:, k, m * P:(m + 1) * P],
                            qin[:, k, :],
                            start=(k == 0), stop=(k == KC - 1))
                    nc.vector.tensor_scalar_add(
                        qt[:, m, sh * SH:(sh + 1) * SH], acc[:], bq_sb[:, m:m + 1])

            wk_sb = w_pool.tile([P, KC, F], f32r, tag="w")
            nc.sync.dma_start(wk_sb[:], wk_d[:].rearrange("(k p) f -> p k f", p=P))
            wv_sb = w_pool.tile([P, KC, F], f32r, tag="w")
            nc.sync.dma_start(wv_sb[:], wv_d[:].rearrange("(k p) f -> p k f", p=P))

            # ---- K and V paths (share the xkv load) ----
            for sh in range(2):
                kin = acts_pool.tile([P, KC, SH], f32r, tag="acts")
                vin = acts_pool.tile([P, KC, SH], f32r, tag="acts")
                for c in range(KC):
                    xr = raw_pool.tile([P, SH], f32, tag="raw")
                    pkr = raw_pool.tile([P, SH], f32, tag="raw")
                    pvr = raw_pool.tile([P, SH], f32, tag="raw")
                    nc.sync.dma_start(
                        xr[:], xkv_d[c * P:(c + 1) * P, sh * SH:(sh + 1) * SH])
                    nc.sync.dma_start(
                        pkr[:], pk_d[c * P:(c + 1) * P, sh * SH:(sh + 1) * SH])
                    nc.sync.dma_start(
                        pvr[:], pv_d[c * P:(c + 1) * P, sh * SH:(sh + 1) * SH])
                    nc.vector.tensor_add(kin[:, c, :], xr[:], pkr[:])
                    nc.vector.tensor_add(vin[:, c, :], xr[:], pvr[:])
                # K^T like Q^T
                for m in range(4):
                    acc = ps.tile([P, SH], f32, tag="ps")
                    for k in range(KC):
                        nc.tensor.matmul(
                            acc[:],
                            wk_sb[:, k, m * P:(m + 1) * P],
                            kin[:, k, :],
                            start=(k == 0), stop=(k == KC - 1))
                    nc.vector.tensor_scalar_add(
                        kt[:, m, sh * SH:(sh + 1) * SH], acc[:], bk_sb[:, m:m + 1])
                # V in natural [s, f] layout: lhsT = vin chunk, rhs = wv chunk.
                for s in range(4):
                    sc = sh * 4 + s
                    acc = ps.tile([P, SH], f32, tag="ps")
                    for k in range(KC):
                        nc.tensor.matmul(
                            acc[:],
                            vin[:, k, s * P:(s + 1) * P],
                            wv_sb[:, k, :],
                            start=(k == 0), stop=(k == KC - 1))
                    nc.vector.tensor_copy(
                        vsb[:, sc, :, 0:HD],
                        acc[:].rearrange("p (h d) -> p h d", d=HD))

            # ---- attention, one head at a time ----
            for h in range(NH):
                po = (h % 2) * HD   # partition offset within the feature chunk
                mq = h // 2         # feature chunk holding this head
                for sh in range(2):
                    pt = p_pool.tile([P, SC, SH], f32r, tag="pbuf")
                    for c in range(SC):
                        lg = ps.tile([P, SH], f32, tag="ps")
                        nc.tensor.matmul(
                            lg[:],
                            kt[po:po + HD, mq, c * P:(c + 1) * P],
                            qt[po:po + HD, mq, sh * SH:(sh + 1) * SH],
                            start=True, stop=True)
                        # P^T = exp(L^T + key_padding_bias), bias per partition.
                        nc.scalar.activation(
                            pt[:, c, :], lg[:],
                            mybir.ActivationFunctionType.Exp,
                            bias=mb_sb[:, c:c + 1])
                    av = ps.tile([P, SH], f32, tag="ps")
                    for c in range(SC):
                        nc.tensor.matmul(
                            av[:HD + 1, :],
                            vsb[:, c, h, :],
                            pt[:, c, :],
                            start=(c == 0), stop=(c == SC - 1))
                    # row HD of av = softmax denominators for these 512 queries
                    rt = small.tile([1, SH], f32r, tag="rt")
                    with nc.allow_low_precision("f32r rounding of softmax denom"):
                        nc.vector.reciprocal(rt[:], av[HD:HD + 1, :])
                    bc = ps.tile([P, SH], f32, tag="ps")
                    nc.tensor.matmul(
                        bc[0:HD, :], ones_sb[:], rt[:], start=True, stop=True)
                    rb = small.tile([HD, SH], f32, tag="rb")
                    nc.vector.tensor_copy(rb[:], bc[0:HD, :])
                    nc.vector.tensor_mul(
                        xt[po:po + HD, mq, sh * SH:(sh + 1) * SH],
                        av[0:HD, :], rb[:])

            # ---- output projection: out^T = sum_h wo_h^T x_h^T (+ bo) ----
            wo_sb = w_pool.tile([P, 4, D], f32r, tag="w")
            nc.sync.dma_start(wo_sb[:], wo_d[:].rearrange("(k p) f -> p k f", p=P))
            for m in range(KC):
                ob = out_pool.tile([P, S], f32, tag="outb")
                for sh in range(2):
                    acc = ps.tile([P, SH], f32, tag="ps")
                    for hp in range(4):
                        nc.tensor.matmul(
                            acc[:],
                            wo_sb[:, hp, m * P:(m + 1) * P],
                            xt[:, hp, sh * SH:(sh + 1) * SH],
                            start=(hp == 0), stop=(hp == 3))
                    nc.vector.tensor_scalar_add(
                        ob[:, sh * SH:(sh + 1) * SH], acc[:], bo_sb[:, m:m + 1])
                nc.sync.dma_start(out_d[m * P:(m + 1) * P, :], ob[:])

    nc.compile()
    return nc


_program = None
_last_in_maps = None


def _get_program():
    global _program
    if _program is None:
        _program = build_program()
    return _program


def kernel(inputs_q, inputs_kv, pos_emb_q, pos_emb_k, pos_emb_v,
           key_padding_mask, wq, bq, wk, bk, wv, bv, wo, bo):
    nc = _get_program()

    wqf = np.asarray(wq, np.float32).reshape(D, H * HD)
    wkf = np.asarray(wk, np.float32).reshape(D, H * HD)
    wvf = np.asarray(wv, np.float32).reshape(D, H * HD)
    wof = np.asarray(wo, np.float32).reshape(H * HD, D)
    bqf = np.asarray(bq, np.float32).reshape(H * HD)
    bkf = np.asarray(bk, np.float32).reshape(H * HD)
    bvf = np.asarray(bv, np.float32).reshape(H * HD)
    bof = np.asarray(bo, np.float32).reshape(D)
    # bv is structurally zero in this problem; it has no cheap slot in the
    # transposed dataflow, so refuse loudly rather than silently drop it.
    assert np.all(bvf == 0.0), "nonzero bv is not supported"

    iq = np.asarray(inputs_q, np.float32)
    ikv = np.asarray(inputs_kv, np.float32)
    pqa = np.asarray(pos_emb_q, np.float32)
    pka = np.asarray(pos_emb_k, np.float32)
    pva = np.asarray(pos_emb_v, np.float32)
    mask = np.asarray(key_padding_mask, np.float32)

    in_maps = []
    for b in range(B):
        xq_t = np.ascontiguousarray(iq[b].T)
        xkv_t = np.ascontiguousarray(ikv[b].T)
        pq_t = np.ascontiguousarray(pqa[b].T)
        pk_t = np.ascontiguousarray(pka[b].T)
        pv_t = np.ascontiguousarray(pva[b].T)
        mb = ((1.0 - mask[b]) * NEG_BIG).astype(np.float32)
        for hg in range(2):
            sl = slice(hg * F, (hg + 1) * F)
            in_maps.append({
                "xq": xq_t, "xkv": xkv_t, "pq": pq_t, "pk": pk_t, "pv": pv_t,
                "wq": np.ascontiguousarray(wqf[:, sl]) * np.float32(1.0 / np.sqrt(HD)),
                "wk": np.ascontiguousarray(wkf[:, sl]),
                "wv": np.ascontiguousarray(wvf[:, sl]),
                "wo": np.ascontiguousarray(wof[sl, :]),
                "bq": np.ascontiguousarray(bqf[sl]) * np.float32(1.0 / np.sqrt(HD)),
                "bk": np.ascontiguousarray(bkf[sl]),
                "bo": bof if hg == 0 else np.zeros_like(bof),
                "mb": mb,
                "vones": np.ones((P, SC, NH), np.float32),
                "ones": np.ones((1, HD), np.float32),
            })

    global _last_in_maps
    _last_in_maps = in_maps
    res = run_bass_kernel_spmd(nc, in_maps, list(range(2 * B)))
    outs = [res.results[i]["out_t"] for i in range(2 * B)]
    out = np.stack([(outs[2 * b] + outs[2 * b + 1]).T for b in range(B)])
    return np.ascontiguousarray(out, dtype=np.float32)
